# revision 25
# baseline (speedup 1.0000x reference)
"""Trainium2 Bass kernel for a quantized Mistral-style SwiGLU MLP.

Reference computation (per token x of dim HIDDEN=4096):
    g = x @ (gate_wq * gate_scale[:, None]).T      # [INTER]
    u = x @ (up_wq   * up_scale[:, None]).T        # [INTER]
    h = silu(g) * u
    y = h @ (down_wq * down_scale[:, None]).T      # [HIDDEN]

Sharding across 8 NeuronCores: DP4 (token groups of 2048) x TP2 (intermediate
shards of 7168).  Each core runs the same SPMD program:
  phase 1: xT resident in SBUF (feature-major [hid, tok]); stream gate/up
           weight tiles; PE matmuls accumulate g,u in PSUM; ACT applies
           silu(gate_scale*g); DVE forms h = (up_scale*u) * silu(...) in bf16;
           h goes to a DRAM bounce buffer.
  phase 2: h streamed back per k-quarter (resident in SBUF); PE matmuls
           against down weight tiles accumulate each quarter's [hid, tok]
           fp32 partial in PSUM, and the 4 partials are summed in DRAM via
           SWDGE accumulate-DMA (output buffers are zero-initialized by the
           SPMD runner).
Host sums the TP pair, applies down_scale, and re-assembles [B, S, HIDDEN].

All weights are fed to the device as exact bf16 integers (values in
[-128,127] are exactly representable); scales stay fp32 and are applied
per-partition on chip (gate/up) or on host (down).
"""

import numpy as np
import ml_dtypes

import concourse.bacc as bacc
import concourse.mybir as mybir
import concourse.tile as tile
from concourse.bass_utils import run_bass_kernel_spmd

BF16 = ml_dtypes.bfloat16
BF = mybir.dt.bfloat16
F32 = mybir.dt.float32

N_CORES = 8
DP, TP = 4, 2
HIDDEN, INTER = 4096, 14336
B, S = 4, 2048

P = 128
FD = 512  # matmul moving free dim (one PSUM bank of fp32)


def dedupe_ldw(nc):
    """Drop PE InstLdweights identical to the previous one when only
    matmuls sit in between (the PE array still holds those weights).
    Only sync-free LDWs are dropped, so semaphore behavior is unchanged."""
    n_drop = 0
    for fn in nc.m.functions:
        for blk in fn.blocks:
            last_key = None
            keep = []
            for inst in blk.instructions:
                if isinstance(inst, mybir.InstLdweights):
                    key = str(inst.ins[0])
                    si = inst.sync_info
                    clean = si is None or (not si.on_wait and not si.on_update)
                    if key == last_key and clean:
                        n_drop += 1
                        continue
                    last_key = key
                elif isinstance(inst, mybir.InstMatmult):
                    pass  # consumes, does not clobber, loaded weights
                elif inst.engine == mybir.EngineType.PE:
                    last_key = None
                keep.append(inst)
            if len(keep) != len(blk.instructions):
                blk.instructions[:] = keep
    return n_drop


def slim_pe_sems(nc):
    """Drop sem-incs from matmuls nobody waits on.

    Tile gives every matmul a +1 update on the PE semaphore, but consumers
    only wait at accumulation-group boundaries (240 distinct wait values vs
    21504 incs here).  Each EVT_SEM write costs ~26ns of PE issue time, so
    keep only the awaited incs (plus the final one) and renumber the waits.
    Safe because PE completes matmuls in program order.
    """
    fn = nc.m.functions[0]
    blocks = list(fn.blocks)
    upd_by = {}
    for blk in blocks:
        for inst in blk.instructions:
            si = inst.sync_info
            if si is None:
                continue
            for u in si.on_update:
                upd_by.setdefault(u.id, set()).add(
                    (type(inst).__name__, u.update_mode, u.update_value))
    cand = {sid for sid, kinds in upd_by.items()
            if kinds == {("InstMatmult", "sem-inc", 1)}}
    n_drop = 0
    for sid in cand:
        incs, waits = [], []
        ok = True
        for blk in blocks:
            for inst in blk.instructions:
                si = inst.sync_info
                if si is None:
                    continue
                for u in si.on_update:
                    if u.id == sid:
                        incs.append(si)
                for w in si.on_wait:
                    if w.id == sid:
                        if w.wait_mode != "sem-ge-imm" or w.wait_reg is not None:
                            ok = False
                        waits.append(w)
        awaited = sorted({w.wait_value for w in waits})
        if not ok or not incs or (awaited and awaited[-1] > len(incs)):
            continue
        keep = set(awaited)
        keep.add(len(incs))
        newval = {v: i + 1 for i, v in enumerate(sorted(keep))}
        for idx, si in enumerate(incs, start=1):
            if idx not in keep:
                si.on_update = [u for u in si.on_update if u.id != sid]
                n_drop += 1
        for w in waits:
            w.wait_value = newval[w.wait_value]
    return n_drop


def build_module(hidden, inter_sh, m, kq_splits=None, loop_order="k",
                 do_dedupe=True, do_slim=True, x_chunked=False,
                 y_split=False, strassen_p1=False):
    """Build the per-core SPMD Bass module.

    hidden:   full hidden dim (contraction of phase 1, output of phase 2)
    inter_sh: this core's intermediate-dim shard
    m:        tokens per core
    kq_splits: phase-2 contraction chunk sizes (in 128-tiles), sum == NO
    x_chunked: load x in KH per-chunk DMAs so matmuls start early
    y_split:  separate y output per kq chunk (host sums) instead of
              SWDGE accumulate-DMA
    """
    KH = hidden // P        # phase-1 contraction chunks
    NO = inter_sh // P      # phase-1 output tiles (inter)
    OH = hidden // P        # phase-2 output tiles (hid)
    MQ = m // FD            # moving passes per psum row
    if kq_splits is None:
        kq_splits = [NO // 4] * 4 if NO % 4 == 0 else [NO]
    assert sum(kq_splits) == NO
    KQ = len(kq_splits)
    KKmax = max(kq_splits)
    assert m % FD == 0

    nc = bacc.Bacc("TRN2", target_bir_lowering=False, debug=False,
                   num_devices=N_CORES)

    xT_d = nc.dram_tensor("xT", [P, KH, m], BF, kind="ExternalInput").ap()
    if not strassen_p1:
        gw_d = nc.dram_tensor("gw", [NO, P, KH * P], BF,
                              kind="ExternalInput").ap()
        uw_d = nc.dram_tensor("uw", [NO, P, KH * P], BF,
                              kind="ExternalInput").ap()
    dw_d = nc.dram_tensor("dw", [OH, P, NO * P], BF, kind="ExternalInput").ap()
    gs_d = nc.dram_tensor("gs", [P, NO], F32, kind="ExternalInput").ap()
    us_d = nc.dram_tensor("us", [P, NO], F32, kind="ExternalInput").ap()
    n_y = KQ * OH if (y_split and KQ > 1) else OH
    y_d = nc.dram_tensor("y", [n_y, P, m], F32, kind="ExternalOutput").ap()

    mult = mybir.AluOpType.mult
    add = mybir.AluOpType.add
    sub = mybir.AluOpType.subtract
    silu = mybir.ActivationFunctionType.Silu

    if strassen_p1:
        # weight-combo input replaces gw/uw: [NO2, 7, P, KH2*P] per g/u
        NO2, KH2 = NO // 2, KH // 2
        TB, JH = m // 512, 256  # token blocks of 512 = two j-halves of 256
        gw_d = nc.dram_tensor("gwS", [NO2, 7, P, KH2 * P], BF,
                              kind="ExternalInput").ap()
        uw_d = nc.dram_tensor("uwS", [NO2, 7, P, KH2 * P], BF,
                              kind="ExternalInput").ap()

    with tile.TileContext(nc) as tc:
        with tc.tile_pool(name="const", bufs=1) as cpool, \
             tc.tile_pool(name="dram", bufs=1, space="DRAM") as dpool:
            gs_sb = cpool.tile([P, NO], F32, tag="gs")
            us_sb = cpool.tile([P, NO], F32, tag="us")
            nc.sync.dma_start(out=gs_sb[:], in_=gs_d[:])
            nc.sync.dma_start(out=us_sb[:], in_=us_d[:])
            h_d = dpool.tile([NO, P, m], BF)

            # ---------------- phase 1 (Strassen): 7/8 of the matmuls ----
            # 2x2x2 Strassen over (inter x hid x tok). x is stored ONLY as
            # its 7 B-side operands (7/4 of the half-matrices); token dim is
            # processed in two halves so those operands fit SBUF.  M1..M7
            # accumulate in a 7x256 PSUM tile (3.5 banks, double-buffered);
            # DVE combines them into C blocks, then silu/gating as usual.
            if strassen_p1:
                NO2, KH2 = NO // 2, KH // 2
                # operand index per M_i: c0..c4 derived, 5=B11, 6=B22
                B_OF_M = [0, 5, 1, 2, 6, 3, 4]
                with tc.tile_pool(name="bcp", bufs=1) as bcp, \
                     tc.tile_pool(name="stp", bufs=2) as stp, \
                     tc.tile_pool(name="wsp", bufs=12) as wsp, \
                     tc.tile_pool(name="sp", bufs=2) as sp, \
                     tc.tile_pool(name="pp", bufs=2, space="PSUM") as pp:
                    for mh in range(2):  # token halves
                        m2 = m // 2
                        mo = mh * m2
                        TB = m2 // 512
                        bc = bcp.tile([P, 7, KH2, TB, 256], BF, tag="bc")
                        # raw blocks straight from DRAM: B11(5), B22(6)
                        xh0 = xT_d[:, 0:KH2, mo:mo + m2].rearrange(
                            "p k (tb j) -> p k tb j", tb=TB)
                        xh1 = xT_d[:, KH2:KH, mo:mo + m2].rearrange(
                            "p k (tb j) -> p k tb j", tb=TB)
                        for k in range(KH2):
                            nc.sync.dma_start(out=bc[:, 5, k],
                                              in_=xh0[:, k, :, 0:256])
                            nc.sync.dma_start(out=bc[:, 6, k],
                                              in_=xh1[:, k, :, 256:512])
                        # derived combos via chunk-pair staging
                        for k in range(KH2):
                            st = stp.tile([P, 2, m2], BF, tag="st")
                            nc.sync.dma_start(out=st[:, 0, :],
                                              in_=xT_d[:, k, mo:mo + m2])
                            nc.sync.dma_start(out=st[:, 1, :],
                                              in_=xT_d[:, KH2 + k,
                                                       mo:mo + m2])
                            s0 = st[:, 0, :].rearrange(
                                "p (tb j) -> p tb j", tb=TB)
                            s1 = st[:, 1, :].rearrange(
                                "p (tb j) -> p tb j", tb=TB)
                            B11 = s0[:, :, 0:256]
                            B12 = s0[:, :, 256:512]
                            B21 = s1[:, :, 0:256]
                            B22 = s1[:, :, 256:512]
                            for ci, (a, b, op) in enumerate([
                                    (B11, B22, add),    # c0: M1
                                    (B12, B22, sub),    # c1: M3
                                    (B21, B11, sub),    # c2: M4
                                    (B11, B12, add),    # c3: M6
                                    (B21, B22, add)]):  # c4: M7
                                nc.vector.tensor_tensor(
                                    out=bc[:, ci, k], in0=a, in1=b, op=op)
                        for o in range(NO2):
                            wtiles = {}
                            for gi, wsrc in ((0, gw_d), (1, uw_d)):
                                for mi in range(7):
                                    wt = wsp.tile([P, KH2 * P], BF,
                                                  tag="ws")
                                    nc.sync.dma_start(out=wt[:],
                                                      in_=wsrc[o, mi])
                                    wtiles[gi, mi] = wt
                            for tb in range(TB):
                                sg = None
                                for gi in (0, 1):
                                    ps = pp.tile([P, 7 * 256], F32,
                                                 tag="ms")
                                    for mi in range(7):
                                        rhs = bc[:, B_OF_M[mi], :, tb, :]
                                        wt = wtiles[gi, mi]
                                        for k in range(KH2):
                                            nc.tensor.matmul(
                                                ps[:, mi*256:(mi+1)*256],
                                                wt[:, k*P:(k+1)*P],
                                                rhs[:, k, :],
                                                start=(k == 0),
                                                stop=(k == KH2 - 1))
                                    M = [ps[:, i*256:(i+1)*256]
                                         for i in range(7)]
                                    cc = sp.tile([P, 1024], F32,
                                                 tag="cg" if gi == 0
                                                 else "cu")
                                    sc = sp.tile([P, 1024], F32, tag="sc")
                                    # DVE may read at most ONE psum
                                    # operand per instruction: stage
                                    # M1/M3/M4 in SBUF scratch first.
                                    t = sc[:, 0:256]      # M1
                                    w = sc[:, 256:512]    # M3
                                    s = sc[:, 512:768]    # M4
                                    u = sc[:, 768:1024]   # M1+M4
                                    nc.vector.tensor_copy(t, M[0])
                                    nc.vector.tensor_copy(w, M[2])
                                    nc.vector.tensor_copy(s, M[3])
                                    tt = nc.vector.tensor_tensor
                                    tt(out=u, in0=t, in1=s, op=add)
                                    tt(out=cc[:, 0:256], in0=u,
                                       in1=M[4], op=sub)
                                    tt(out=cc[:, 0:256], in0=cc[:, 0:256],
                                       in1=M[6], op=add)  # C11=M1+M4-M5+M7
                                    tt(out=cc[:, 256:512], in0=w,
                                       in1=M[4], op=add)  # C12=M3+M5
                                    tt(out=cc[:, 512:768], in0=s,
                                       in1=M[1], op=add)  # C21=M2+M4
                                    tt(out=cc[:, 768:1024], in0=t,
                                       in1=M[1], op=sub)
                                    tt(out=cc[:, 768:1024],
                                       in0=cc[:, 768:1024], in1=w,
                                       op=add)
                                    tt(out=cc[:, 768:1024],
                                       in0=cc[:, 768:1024], in1=M[5],
                                       op=add)            # C22=M1-M2+M3+M6
                                    lo = mo + tb * 512
                                    if gi == 0:
                                        sg = sp.tile([P, 1024], BF,
                                                     tag="sg")
                                        nc.scalar.activation(
                                            sg[:, 0:512], cc[:, 0:512],
                                            silu,
                                            scale=gs_sb[:, o:o + 1])
                                        nc.scalar.activation(
                                            sg[:, 512:1024],
                                            cc[:, 512:1024], silu,
                                            scale=gs_sb[:,
                                                        o+NO2:o+NO2+1])
                                    else:
                                        hb = sp.tile([P, 1024], BF,
                                                     tag="hb")
                                        nc.vector.scalar_tensor_tensor(
                                            hb[:, 0:512], cc[:, 0:512],
                                            us_sb[:, o:o + 1],
                                            sg[:, 0:512], mult, mult)
                                        nc.vector.scalar_tensor_tensor(
                                            hb[:, 512:1024],
                                            cc[:, 512:1024],
                                            us_sb[:, o+NO2:o+NO2+1],
                                            sg[:, 512:1024], mult, mult)
                                        nc.sync.dma_start(
                                            out=h_d[o][:, lo:lo + 512],
                                            in_=hb[:, 0:512])
                                        nc.sync.dma_start(
                                            out=h_d[o + NO2][:,
                                                             lo:lo + 512],
                                            in_=hb[:, 512:1024])

            # ---------------- phase 1: h = silu(gs*g) * (us*u) ----------
            if not strassen_p1:
                with tc.tile_pool(name="xp", bufs=1) as xp, \
                     tc.tile_pool(name="wp", bufs=2) as wp, \
                     tc.tile_pool(name="sp", bufs=2) as sp, \
                     tc.tile_pool(name="pp", bufs=1, space="PSUM") as pp:
                    x_sb = xp.tile([P, KH, m], BF, tag="x")
                    if x_chunked:
                        for k in range(KH):
                            nc.sync.dma_start(out=x_sb[:, k, :],
                                              in_=xT_d[:, k, :])
                    else:
                        nc.sync.dma_start(out=x_sb[:], in_=xT_d[:])
                    for o in range(NO):
                        gwt = wp.tile([P, KH * P], BF, tag="gw")
                        uwt = wp.tile([P, KH * P], BF, tag="uw")
                        nc.sync.dma_start(out=gwt[:], in_=gw_d[o])
                        nc.sync.dma_start(out=uwt[:], in_=uw_d[o])
                        pg = pp.tile([P, m], F32, tag="pg")
                        pu = pp.tile([P, m], F32, tag="pu")
                        # k-outer: each weight tile is held across the 4
                        # moving passes, and dedupe_ldw() drops the 3
                        # redundant LDWEIGHTS per group.
                        def loops():
                            if loop_order == "k":
                                return [(k, q) for k in range(KH)
                                        for q in range(MQ)]
                            return [(k, q) for q in range(MQ)
                                    for k in range(KH)]
                        for k, q in loops():
                            nc.tensor.matmul(
                                pg[:, q * FD:(q + 1) * FD],
                                gwt[:, k * P:(k + 1) * P],
                                x_sb[:, k, q * FD:(q + 1) * FD],
                                start=(k == 0), stop=(k == KH - 1))
                        for k, q in loops():
                            nc.tensor.matmul(
                                pu[:, q * FD:(q + 1) * FD],
                                uwt[:, k * P:(k + 1) * P],
                                x_sb[:, k, q * FD:(q + 1) * FD],
                                start=(k == 0), stop=(k == KH - 1))
                        sg = sp.tile([P, m], BF, tag="sg")
                        nc.scalar.activation(sg[:], pg[:], silu,
                                             scale=gs_sb[:, o:o + 1])
                        hb = sp.tile([P, m], BF, tag="hb")
                        nc.vector.scalar_tensor_tensor(
                            hb[:], pu[:], us_sb[:, o:o + 1], sg[:],
                            mult, mult)
                        nc.sync.dma_start(out=h_d[o], in_=hb[:])

            # ---- phase 2: y += h[kq] @ down[kq], DMA-accumulated over kq ----
            with tc.tile_pool(name="hqp", bufs=2) as hqp, \
                 tc.tile_pool(name="dwp", bufs=2) as dwp, \
                 tc.tile_pool(name="yop", bufs=2) as yop, \
                 tc.tile_pool(name="pp2", bufs=2, space="PSUM") as pp2:
                off = 0
                for kq, KK in enumerate(kq_splits):
                    hq = hqp.tile([P, KKmax, m], BF, tag="hq")
                    for kk in range(KK):
                        nc.sync.dma_start(out=hq[:, kk, :],
                                          in_=h_d[off + kk])
                    for o in range(OH):
                        dwt = dwp.tile([P, KKmax * P], BF, tag="dw")
                        nc.sync.dma_start(
                            out=dwt[:, :KK * P],
                            in_=dw_d[o][:, off * P:(off + KK) * P])
                        py = pp2.tile([P, m], F32, tag="py")
                        if loop_order == "k":
                            kqs = [(kk, q) for kk in range(KK)
                                   for q in range(MQ)]
                        else:
                            kqs = [(kk, q) for q in range(MQ)
                                   for kk in range(KK)]
                        for kk, q in kqs:
                            nc.tensor.matmul(
                                py[:, q * FD:(q + 1) * FD],
                                dwt[:, kk * P:(kk + 1) * P],
                                hq[:, kk, q * FD:(q + 1) * FD],
                                start=(kk == 0), stop=(kk == KK - 1))
                        yo = yop.tile([P, m], F32, tag="yo")
                        nc.vector.tensor_copy(yo[:], py[:])
                        if KQ == 1:
                            nc.sync.dma_start(out=y_d[o], in_=yo[:])
                        elif y_split:
                            nc.sync.dma_start(out=y_d[kq * OH + o],
                                              in_=yo[:])
                        else:
                            # accumulate partials straight into DRAM (SWDGE);
                            # output buffers are zero-initialized by the runner
                            nc.gpsimd.dma_start(out=y_d[o], in_=yo[:],
                                                accum_op=mybir.AluOpType.add)
                    off += KK

    if do_dedupe:
        dedupe_ldw(nc)
    if do_slim:
        slim_pe_sems(nc)
    nc.compile()
    return nc


def _pack_w(wm, no, kh):
    """[no*P, kh*P] weight matrix -> [no, P, kh*P] lhsT tile pack."""
    return np.ascontiguousarray(
        wm.reshape(no, P, kh, P).transpose(0, 3, 2, 1)).reshape(
            no, P, kh * P)


def _strassen_pack(wq, no2, kh2):
    """7 Strassen A-side operand packs: [no2, 7, P, kh2*P] bf16.
    Values are sums/differences of int8-valued weights (|.| <= 255),
    exactly representable in bf16."""
    n, h = wq.shape
    A11 = wq[:n // 2, :h // 2]
    A12 = wq[:n // 2, h // 2:]
    A21 = wq[n // 2:, :h // 2]
    A22 = wq[n // 2:, h // 2:]
    S = [A11 + A22, A21 + A22, A11, A22, A11 + A12, A21 - A11, A12 - A22]
    out = np.stack([_pack_w(s, no2, kh2) for s in S], axis=1)
    return out.astype(BF16)


def prep_core_inputs(x_flat, gate_wq, gate_scale, up_wq, up_scale, down_wq,
                     hidden, inter, dp, tp, kq_splits=None,
                     strassen_p1=None):
    """Shard + repack full inputs into per-core input maps (list of dicts)."""
    if strassen_p1 is None:
        strassen_p1 = CONFIG["strassen_p1"]
    n_tok = x_flat.shape[0]
    m = n_tok // dp
    inter_sh = inter // tp
    KH = hidden // P
    NO = inter_sh // P
    OH = hidden // P

    # per-TP-shard weight packs (shared by all DP groups)
    packs = []
    for s in range(tp):
        lo, hi = s * inter_sh, (s + 1) * inter_sh
        dq = down_wq[:, lo:hi].astype(BF16)
        # [o,c,j,p] -> [o,p,j,c] -> [OH, P, NO*P]
        dw = np.ascontiguousarray(
            dq.reshape(OH, P, NO, P).transpose(0, 3, 2, 1)).reshape(OH, P, NO * P)
        gs = np.ascontiguousarray(gate_scale[lo:hi].reshape(NO, P).T)
        us = np.ascontiguousarray(up_scale[lo:hi].reshape(NO, P).T)
        if strassen_p1:
            gf = gate_wq[lo:hi].astype(np.float32)
            uf = up_wq[lo:hi].astype(np.float32)
            gwS = _strassen_pack(gf, NO // 2, KH // 2)
            uwS = _strassen_pack(uf, NO // 2, KH // 2)
            packs.append(dict(gwS=gwS, uwS=uwS, dw=dw, gs=gs, us=us))
        else:
            gw = _pack_w(gate_wq[lo:hi].astype(BF16), NO, KH)
            uw = _pack_w(up_wq[lo:hi].astype(BF16), NO, KH)
            packs.append(dict(gw=gw, uw=uw, dw=dw, gs=gs, us=us))

    in_maps = []
    for g in range(dp):
        xg = x_flat[g * m:(g + 1) * m]  # [m, hidden]
        xT = np.ascontiguousarray(xg.T.astype(BF16)).reshape(P * KH, m)
        # [hidden, m] with hidden = k*P + p -> [P, KH, m]
        xT = np.ascontiguousarray(
            xT.reshape(KH, P, m).transpose(1, 0, 2))
        for s in range(tp):
            in_maps.append({"xT": xT, **packs[s]})
    return in_maps


CONFIG = dict(loop_order="k", do_dedupe=True, do_slim=True,
              x_chunked=True, y_split=False, strassen_p1=True,
              kq_splits=None)

_NC_CACHE = {}


def _get_module():
    key = str(sorted(CONFIG.items()))
    if key not in _NC_CACHE:
        _NC_CACHE[key] = build_module(HIDDEN, INTER // TP, (B * S) // DP,
                                      **CONFIG)
    return _NC_CACHE[key]


def kernel(x, gate_wq, gate_scale, up_wq, up_scale, down_wq, down_scale,
           _return_results=False):
    x = np.asarray(x)
    x_flat = x.reshape(B * S, HIDDEN)
    in_maps = prep_core_inputs(
        x_flat, np.asarray(gate_wq), np.asarray(gate_scale),
        np.asarray(up_wq), np.asarray(up_scale), np.asarray(down_wq),
        HIDDEN, INTER, DP, TP)

    nc = _get_module()
    res = run_bass_kernel_spmd(nc, in_maps, list(range(N_CORES)))

    m = (B * S) // DP
    OH = HIDDEN // P
    y = np.empty((B * S, HIDDEN), np.float32)
    ds = np.asarray(down_scale).astype(np.float32)
    for g in range(DP):
        acc = None
        for s in range(TP):
            part = res.results[g * TP + s]["y"]  # [(KQ*)OH, P, m]
            if part.shape[0] != OH:              # y_split: sum kq partials
                part = part.reshape(-1, OH, P, m).sum(axis=0)
            acc = part if acc is None else acc + part
        # [OH, P, m] -> [hidden, m] -> [m, hidden]
        y[g * m:(g + 1) * m] = acc.reshape(HIDDEN, m).T
    y *= ds[None, :]
    out = y.reshape(B, S, HIDDEN)
    if _return_results:
        return out, res
    return out



# revision 26
# speedup vs baseline: 2.1245x; 2.1245x over previous
"""Trainium2 Bass kernel for a quantized Mistral-style SwiGLU MLP.

Reference computation (per token x of dim HIDDEN=4096):
    g = x @ (gate_wq * gate_scale[:, None]).T      # [INTER]
    u = x @ (up_wq   * up_scale[:, None]).T        # [INTER]
    h = silu(g) * u
    y = h @ (down_wq * down_scale[:, None]).T      # [HIDDEN]

Sharding across 8 NeuronCores: DP4 (token groups of 2048) x TP2 (intermediate
shards of 7168).  Each core runs the same SPMD program:
  phase 1 (one-level Strassen, 7/8 the matmul work): weights are shipped as
           the 7 Strassen A-operands (host-built, |sums| <= 255 so bf16 is
           exact); x is stored in SBUF only as its 7 B-operands (built by
           DVE/DMA per token half); M1..M7 accumulate in a [P, 7*256] PSUM
           tile (double-buffered); DVE combines them into C blocks (max one
           PSUM operand per instruction); ACT applies silu(gate_scale*g);
           DVE forms h = (up_scale*u) * silu(...) in bf16; h goes to a DRAM
           bounce buffer.  Rel err ~0.0055 vs 0.0033 dense (gate 2e-2).
  phase 2: h streamed back per k-quarter (resident in SBUF); PE matmuls
           against down weight tiles accumulate each quarter's [hid, tok]
           fp32 partial in PSUM, and the 4 partials are summed in DRAM via
           SWDGE accumulate-DMA (output buffers are zero-initialized by the
           SPMD runner).
Host sums the TP pair, applies down_scale, and re-assembles [B, S, HIDDEN].

Why Strassen: the chip power-throttles PE to ~2.0 GHz when all 8 cores run
sustained matmuls (1-core measures 4.70 ms = the 2.4 GHz cost model; 8-core
5.5-5.8 ms), so the only way past the throttled roofline is fewer PE cycles.
LDWEIGHTS dedup and sem-inc thinning measured neutral (hidden by the PE's
reorder window); they are kept as cheap IR cleanups.

All weights are fed to the device as exact bf16 integers (values in
[-128,127] are exactly representable); scales stay fp32 and are applied
per-partition on chip (gate/up) or on host (down).
"""

import numpy as np
import ml_dtypes

import concourse.bacc as bacc
import concourse.mybir as mybir
import concourse.tile as tile
from concourse.bass_utils import run_bass_kernel_spmd

BF16 = ml_dtypes.bfloat16
BF = mybir.dt.bfloat16
F32 = mybir.dt.float32

N_CORES = 8
DP, TP = 4, 2
HIDDEN, INTER = 4096, 14336
B, S = 4, 2048

P = 128
FD = 512  # matmul moving free dim (one PSUM bank of fp32)


def dedupe_ldw(nc):
    """Drop PE InstLdweights identical to the previous one when only
    matmuls sit in between (the PE array still holds those weights).
    Only sync-free LDWs are dropped, so semaphore behavior is unchanged."""
    n_drop = 0
    for fn in nc.m.functions:
        for blk in fn.blocks:
            last_key = None
            keep = []
            for inst in blk.instructions:
                if isinstance(inst, mybir.InstLdweights):
                    key = str(inst.ins[0])
                    si = inst.sync_info
                    clean = si is None or (not si.on_wait and not si.on_update)
                    if key == last_key and clean:
                        n_drop += 1
                        continue
                    last_key = key
                elif isinstance(inst, mybir.InstMatmult):
                    pass  # consumes, does not clobber, loaded weights
                elif inst.engine == mybir.EngineType.PE:
                    last_key = None
                keep.append(inst)
            if len(keep) != len(blk.instructions):
                blk.instructions[:] = keep
    return n_drop


def slim_pe_sems(nc):
    """Drop sem-incs from matmuls nobody waits on.

    Tile gives every matmul a +1 update on the PE semaphore, but consumers
    only wait at accumulation-group boundaries (240 distinct wait values vs
    21504 incs here).  Each EVT_SEM write costs ~26ns of PE issue time, so
    keep only the awaited incs (plus the final one) and renumber the waits.
    Safe because PE completes matmuls in program order.
    """
    fn = nc.m.functions[0]
    blocks = list(fn.blocks)
    upd_by = {}
    for blk in blocks:
        for inst in blk.instructions:
            si = inst.sync_info
            if si is None:
                continue
            for u in si.on_update:
                upd_by.setdefault(u.id, set()).add(
                    (type(inst).__name__, u.update_mode, u.update_value))
    cand = {sid for sid, kinds in upd_by.items()
            if kinds == {("InstMatmult", "sem-inc", 1)}}
    n_drop = 0
    for sid in cand:
        incs, waits = [], []
        ok = True
        for blk in blocks:
            for inst in blk.instructions:
                si = inst.sync_info
                if si is None:
                    continue
                for u in si.on_update:
                    if u.id == sid:
                        incs.append(si)
                for w in si.on_wait:
                    if w.id == sid:
                        if w.wait_mode != "sem-ge-imm" or w.wait_reg is not None:
                            ok = False
                        waits.append(w)
        awaited = sorted({w.wait_value for w in waits})
        if not ok or not incs or (awaited and awaited[-1] > len(incs)):
            continue
        keep = set(awaited)
        keep.add(len(incs))
        newval = {v: i + 1 for i, v in enumerate(sorted(keep))}
        for idx, si in enumerate(incs, start=1):
            if idx not in keep:
                si.on_update = [u for u in si.on_update if u.id != sid]
                n_drop += 1
        for w in waits:
            w.wait_value = newval[w.wait_value]
    return n_drop


def build_module(hidden, inter_sh, m, kq_splits=None, loop_order="k",
                 do_dedupe=True, do_slim=True, x_chunked=False,
                 y_split=False, strassen_p1=False):
    """Build the per-core SPMD Bass module.

    hidden:   full hidden dim (contraction of phase 1, output of phase 2)
    inter_sh: this core's intermediate-dim shard
    m:        tokens per core
    kq_splits: phase-2 contraction chunk sizes (in 128-tiles), sum == NO
    x_chunked: load x in KH per-chunk DMAs so matmuls start early
    y_split:  separate y output per kq chunk (host sums) instead of
              SWDGE accumulate-DMA
    """
    KH = hidden // P        # phase-1 contraction chunks
    NO = inter_sh // P      # phase-1 output tiles (inter)
    OH = hidden // P        # phase-2 output tiles (hid)
    MQ = m // FD            # moving passes per psum row
    if kq_splits is None:
        kq_splits = [NO // 4] * 4 if NO % 4 == 0 else [NO]
    assert sum(kq_splits) == NO
    KQ = len(kq_splits)
    KKmax = max(kq_splits)
    assert m % FD == 0

    nc = bacc.Bacc("TRN2", target_bir_lowering=False, debug=False,
                   num_devices=N_CORES)

    xT_d = nc.dram_tensor("xT", [P, KH, m], BF, kind="ExternalInput").ap()
    if not strassen_p1:
        gw_d = nc.dram_tensor("gw", [NO, P, KH * P], BF,
                              kind="ExternalInput").ap()
        uw_d = nc.dram_tensor("uw", [NO, P, KH * P], BF,
                              kind="ExternalInput").ap()
    dw_d = nc.dram_tensor("dw", [OH, P, NO * P], BF, kind="ExternalInput").ap()
    gs_d = nc.dram_tensor("gs", [P, NO], F32, kind="ExternalInput").ap()
    us_d = nc.dram_tensor("us", [P, NO], F32, kind="ExternalInput").ap()
    n_y = KQ * OH if (y_split and KQ > 1) else OH
    y_d = nc.dram_tensor("y", [n_y, P, m], F32, kind="ExternalOutput").ap()

    mult = mybir.AluOpType.mult
    add = mybir.AluOpType.add
    sub = mybir.AluOpType.subtract
    silu = mybir.ActivationFunctionType.Silu

    if strassen_p1:
        # weight-combo input replaces gw/uw: [NO2, 7, P, KH2*P] per g/u
        NO2, KH2 = NO // 2, KH // 2
        TB, JH = m // 512, 256  # token blocks of 512 = two j-halves of 256
        gw_d = nc.dram_tensor("gwS", [NO2, 7, P, KH2 * P], BF,
                              kind="ExternalInput").ap()
        uw_d = nc.dram_tensor("uwS", [NO2, 7, P, KH2 * P], BF,
                              kind="ExternalInput").ap()

    with tile.TileContext(nc) as tc:
        with tc.tile_pool(name="const", bufs=1) as cpool, \
             tc.tile_pool(name="dram", bufs=1, space="DRAM") as dpool:
            gs_sb = cpool.tile([P, NO], F32, tag="gs")
            us_sb = cpool.tile([P, NO], F32, tag="us")
            nc.sync.dma_start(out=gs_sb[:], in_=gs_d[:])
            nc.sync.dma_start(out=us_sb[:], in_=us_d[:])
            h_d = dpool.tile([NO, P, m], BF)

            # ---------------- phase 1 (Strassen): 7/8 of the matmuls ----
            # 2x2x2 Strassen over (inter x hid x tok). x is stored ONLY as
            # its 7 B-side operands (7/4 of the half-matrices); token dim is
            # processed in two halves so those operands fit SBUF.  M1..M7
            # accumulate in a 7x256 PSUM tile (3.5 banks, double-buffered);
            # DVE combines them into C blocks, then silu/gating as usual.
            if strassen_p1:
                NO2, KH2 = NO // 2, KH // 2
                # operand index per M_i: c0..c4 derived, 5=B11, 6=B22
                B_OF_M = [0, 5, 1, 2, 6, 3, 4]
                with tc.tile_pool(name="bcp", bufs=1) as bcp, \
                     tc.tile_pool(name="stp", bufs=2) as stp, \
                     tc.tile_pool(name="wsp", bufs=12) as wsp, \
                     tc.tile_pool(name="sp", bufs=2) as sp, \
                     tc.tile_pool(name="pp", bufs=2, space="PSUM") as pp:
                    for mh in range(2):  # token halves
                        m2 = m // 2
                        mo = mh * m2
                        TB = m2 // 512
                        bc = bcp.tile([P, 7, KH2, TB, 256], BF, tag="bc")
                        # raw blocks straight from DRAM: B11(5), B22(6)
                        xh0 = xT_d[:, 0:KH2, mo:mo + m2].rearrange(
                            "p k (tb j) -> p k tb j", tb=TB)
                        xh1 = xT_d[:, KH2:KH, mo:mo + m2].rearrange(
                            "p k (tb j) -> p k tb j", tb=TB)
                        for k in range(KH2):
                            nc.sync.dma_start(out=bc[:, 5, k],
                                              in_=xh0[:, k, :, 0:256])
                            nc.sync.dma_start(out=bc[:, 6, k],
                                              in_=xh1[:, k, :, 256:512])
                        # derived combos via chunk-pair staging
                        for k in range(KH2):
                            st = stp.tile([P, 2, m2], BF, tag="st")
                            nc.sync.dma_start(out=st[:, 0, :],
                                              in_=xT_d[:, k, mo:mo + m2])
                            nc.sync.dma_start(out=st[:, 1, :],
                                              in_=xT_d[:, KH2 + k,
                                                       mo:mo + m2])
                            s0 = st[:, 0, :].rearrange(
                                "p (tb j) -> p tb j", tb=TB)
                            s1 = st[:, 1, :].rearrange(
                                "p (tb j) -> p tb j", tb=TB)
                            B11 = s0[:, :, 0:256]
                            B12 = s0[:, :, 256:512]
                            B21 = s1[:, :, 0:256]
                            B22 = s1[:, :, 256:512]
                            for ci, (a, b, op) in enumerate([
                                    (B11, B22, add),    # c0: M1
                                    (B12, B22, sub),    # c1: M3
                                    (B21, B11, sub),    # c2: M4
                                    (B11, B12, add),    # c3: M6
                                    (B21, B22, add)]):  # c4: M7
                                nc.vector.tensor_tensor(
                                    out=bc[:, ci, k], in0=a, in1=b, op=op)
                        for o in range(NO2):
                            wtiles = {}
                            for gi, wsrc in ((0, gw_d), (1, uw_d)):
                                for mi in range(7):
                                    wt = wsp.tile([P, KH2 * P], BF,
                                                  tag="ws")
                                    nc.sync.dma_start(out=wt[:],
                                                      in_=wsrc[o, mi])
                                    wtiles[gi, mi] = wt
                            for tb in range(TB):
                                sg = None
                                for gi in (0, 1):
                                    ps = pp.tile([P, 7 * 256], F32,
                                                 tag="ms")
                                    for mi in range(7):
                                        rhs = bc[:, B_OF_M[mi], :, tb, :]
                                        wt = wtiles[gi, mi]
                                        for k in range(KH2):
                                            nc.tensor.matmul(
                                                ps[:, mi*256:(mi+1)*256],
                                                wt[:, k*P:(k+1)*P],
                                                rhs[:, k, :],
                                                start=(k == 0),
                                                stop=(k == KH2 - 1))
                                    M = [ps[:, i*256:(i+1)*256]
                                         for i in range(7)]
                                    cc = sp.tile([P, 1024], F32,
                                                 tag="cg" if gi == 0
                                                 else "cu")
                                    sc = sp.tile([P, 1024], F32, tag="sc")
                                    # DVE may read at most ONE psum
                                    # operand per instruction: stage
                                    # M1/M3/M4 in SBUF scratch first.
                                    t = sc[:, 0:256]      # M1
                                    w = sc[:, 256:512]    # M3
                                    s = sc[:, 512:768]    # M4
                                    u = sc[:, 768:1024]   # M1+M4
                                    nc.vector.tensor_copy(t, M[0])
                                    nc.vector.tensor_copy(w, M[2])
                                    nc.vector.tensor_copy(s, M[3])
                                    tt = nc.vector.tensor_tensor
                                    tt(out=u, in0=t, in1=s, op=add)
                                    tt(out=cc[:, 0:256], in0=u,
                                       in1=M[4], op=sub)
                                    tt(out=cc[:, 0:256], in0=cc[:, 0:256],
                                       in1=M[6], op=add)  # C11=M1+M4-M5+M7
                                    tt(out=cc[:, 256:512], in0=w,
                                       in1=M[4], op=add)  # C12=M3+M5
                                    tt(out=cc[:, 512:768], in0=s,
                                       in1=M[1], op=add)  # C21=M2+M4
                                    tt(out=cc[:, 768:1024], in0=t,
                                       in1=M[1], op=sub)
                                    tt(out=cc[:, 768:1024],
                                       in0=cc[:, 768:1024], in1=w,
                                       op=add)
                                    tt(out=cc[:, 768:1024],
                                       in0=cc[:, 768:1024], in1=M[5],
                                       op=add)            # C22=M1-M2+M3+M6
                                    lo = mo + tb * 512
                                    if gi == 0:
                                        sg = sp.tile([P, 1024], BF,
                                                     tag="sg")
                                        nc.scalar.activation(
                                            sg[:, 0:512], cc[:, 0:512],
                                            silu,
                                            scale=gs_sb[:, o:o + 1])
                                        nc.scalar.activation(
                                            sg[:, 512:1024],
                                            cc[:, 512:1024], silu,
                                            scale=gs_sb[:,
                                                        o+NO2:o+NO2+1])
                                    else:
                                        hb = sp.tile([P, 1024], BF,
                                                     tag="hb")
                                        nc.vector.scalar_tensor_tensor(
                                            hb[:, 0:512], cc[:, 0:512],
                                            us_sb[:, o:o + 1],
                                            sg[:, 0:512], mult, mult)
                                        nc.vector.scalar_tensor_tensor(
                                            hb[:, 512:1024],
                                            cc[:, 512:1024],
                                            us_sb[:, o+NO2:o+NO2+1],
                                            sg[:, 512:1024], mult, mult)
                                        nc.sync.dma_start(
                                            out=h_d[o][:, lo:lo + 512],
                                            in_=hb[:, 0:512])
                                        nc.sync.dma_start(
                                            out=h_d[o + NO2][:,
                                                             lo:lo + 512],
                                            in_=hb[:, 512:1024])

            # ---------------- phase 1: h = silu(gs*g) * (us*u) ----------
            if not strassen_p1:
                with tc.tile_pool(name="xp", bufs=1) as xp, \
                     tc.tile_pool(name="wp", bufs=2) as wp, \
                     tc.tile_pool(name="sp", bufs=2) as sp, \
                     tc.tile_pool(name="pp", bufs=1, space="PSUM") as pp:
                    x_sb = xp.tile([P, KH, m], BF, tag="x")
                    if x_chunked:
                        for k in range(KH):
                            nc.sync.dma_start(out=x_sb[:, k, :],
                                              in_=xT_d[:, k, :])
                    else:
                        nc.sync.dma_start(out=x_sb[:], in_=xT_d[:])
                    for o in range(NO):
                        gwt = wp.tile([P, KH * P], BF, tag="gw")
                        uwt = wp.tile([P, KH * P], BF, tag="uw")
                        nc.sync.dma_start(out=gwt[:], in_=gw_d[o])
                        nc.sync.dma_start(out=uwt[:], in_=uw_d[o])
                        pg = pp.tile([P, m], F32, tag="pg")
                        pu = pp.tile([P, m], F32, tag="pu")
                        # k-outer: each weight tile is held across the 4
                        # moving passes, and dedupe_ldw() drops the 3
                        # redundant LDWEIGHTS per group.
                        def loops():
                            if loop_order == "k":
                                return [(k, q) for k in range(KH)
                                        for q in range(MQ)]
                            return [(k, q) for q in range(MQ)
                                    for k in range(KH)]
                        for k, q in loops():
                            nc.tensor.matmul(
                                pg[:, q * FD:(q + 1) * FD],
                                gwt[:, k * P:(k + 1) * P],
                                x_sb[:, k, q * FD:(q + 1) * FD],
                                start=(k == 0), stop=(k == KH - 1))
                        for k, q in loops():
                            nc.tensor.matmul(
                                pu[:, q * FD:(q + 1) * FD],
                                uwt[:, k * P:(k + 1) * P],
                                x_sb[:, k, q * FD:(q + 1) * FD],
                                start=(k == 0), stop=(k == KH - 1))
                        sg = sp.tile([P, m], BF, tag="sg")
                        nc.scalar.activation(sg[:], pg[:], silu,
                                             scale=gs_sb[:, o:o + 1])
                        hb = sp.tile([P, m], BF, tag="hb")
                        nc.vector.scalar_tensor_tensor(
                            hb[:], pu[:], us_sb[:, o:o + 1], sg[:],
                            mult, mult)
                        nc.sync.dma_start(out=h_d[o], in_=hb[:])

            # ---- phase 2: y += h[kq] @ down[kq], DMA-accumulated over kq ----
            with tc.tile_pool(name="hqp", bufs=2) as hqp, \
                 tc.tile_pool(name="dwp", bufs=2) as dwp, \
                 tc.tile_pool(name="yop", bufs=2) as yop, \
                 tc.tile_pool(name="pp2", bufs=2, space="PSUM") as pp2:
                off = 0
                for kq, KK in enumerate(kq_splits):
                    hq = hqp.tile([P, KKmax, m], BF, tag="hq")
                    for kk in range(KK):
                        nc.sync.dma_start(out=hq[:, kk, :],
                                          in_=h_d[off + kk])
                    for o in range(OH):
                        dwt = dwp.tile([P, KKmax * P], BF, tag="dw")
                        nc.sync.dma_start(
                            out=dwt[:, :KK * P],
                            in_=dw_d[o][:, off * P:(off + KK) * P])
                        py = pp2.tile([P, m], F32, tag="py")
                        if loop_order == "k":
                            kqs = [(kk, q) for kk in range(KK)
                                   for q in range(MQ)]
                        else:
                            kqs = [(kk, q) for q in range(MQ)
                                   for kk in range(KK)]
                        for kk, q in kqs:
                            nc.tensor.matmul(
                                py[:, q * FD:(q + 1) * FD],
                                dwt[:, kk * P:(kk + 1) * P],
                                hq[:, kk, q * FD:(q + 1) * FD],
                                start=(kk == 0), stop=(kk == KK - 1))
                        yo = yop.tile([P, m], F32, tag="yo")
                        nc.vector.tensor_copy(yo[:], py[:])
                        if KQ == 1:
                            nc.sync.dma_start(out=y_d[o], in_=yo[:])
                        elif y_split:
                            nc.sync.dma_start(out=y_d[kq * OH + o],
                                              in_=yo[:])
                        else:
                            # accumulate partials straight into DRAM (SWDGE);
                            # output buffers are zero-initialized by the runner
                            nc.gpsimd.dma_start(out=y_d[o], in_=yo[:],
                                                accum_op=mybir.AluOpType.add)
                    off += KK

    if do_dedupe:
        dedupe_ldw(nc)
    if do_slim:
        slim_pe_sems(nc)
    nc.compile()
    return nc


def _pack_w(wm, no, kh):
    """[no*P, kh*P] weight matrix -> [no, P, kh*P] lhsT tile pack."""
    return np.ascontiguousarray(
        wm.reshape(no, P, kh, P).transpose(0, 3, 2, 1)).reshape(
            no, P, kh * P)


def _strassen_pack(wq, no2, kh2):
    """7 Strassen A-side operand packs: [no2, 7, P, kh2*P] bf16.
    Values are sums/differences of int8-valued weights (|.| <= 255),
    exactly representable in bf16."""
    n, h = wq.shape
    A11 = wq[:n // 2, :h // 2]
    A12 = wq[:n // 2, h // 2:]
    A21 = wq[n // 2:, :h // 2]
    A22 = wq[n // 2:, h // 2:]
    S = [A11 + A22, A21 + A22, A11, A22, A11 + A12, A21 - A11, A12 - A22]
    out = np.stack([_pack_w(s, no2, kh2) for s in S], axis=1)
    return out.astype(BF16)


def prep_core_inputs(x_flat, gate_wq, gate_scale, up_wq, up_scale, down_wq,
                     hidden, inter, dp, tp, kq_splits=None,
                     strassen_p1=None):
    """Shard + repack full inputs into per-core input maps (list of dicts)."""
    if strassen_p1 is None:
        strassen_p1 = CONFIG["strassen_p1"]
    n_tok = x_flat.shape[0]
    m = n_tok // dp
    inter_sh = inter // tp
    KH = hidden // P
    NO = inter_sh // P
    OH = hidden // P

    # per-TP-shard weight packs (shared by all DP groups)
    packs = []
    for s in range(tp):
        lo, hi = s * inter_sh, (s + 1) * inter_sh
        dq = down_wq[:, lo:hi].astype(BF16)
        # [o,c,j,p] -> [o,p,j,c] -> [OH, P, NO*P]
        dw = np.ascontiguousarray(
            dq.reshape(OH, P, NO, P).transpose(0, 3, 2, 1)).reshape(OH, P, NO * P)
        gs = np.ascontiguousarray(gate_scale[lo:hi].reshape(NO, P).T)
        us = np.ascontiguousarray(up_scale[lo:hi].reshape(NO, P).T)
        if strassen_p1:
            gf = gate_wq[lo:hi].astype(np.float32)
            uf = up_wq[lo:hi].astype(np.float32)
            gwS = _strassen_pack(gf, NO // 2, KH // 2)
            uwS = _strassen_pack(uf, NO // 2, KH // 2)
            packs.append(dict(gwS=gwS, uwS=uwS, dw=dw, gs=gs, us=us))
        else:
            gw = _pack_w(gate_wq[lo:hi].astype(BF16), NO, KH)
            uw = _pack_w(up_wq[lo:hi].astype(BF16), NO, KH)
            packs.append(dict(gw=gw, uw=uw, dw=dw, gs=gs, us=us))

    in_maps = []
    for g in range(dp):
        xg = x_flat[g * m:(g + 1) * m]  # [m, hidden]
        xT = np.ascontiguousarray(xg.T.astype(BF16)).reshape(P * KH, m)
        # [hidden, m] with hidden = k*P + p -> [P, KH, m]
        xT = np.ascontiguousarray(
            xT.reshape(KH, P, m).transpose(1, 0, 2))
        for s in range(tp):
            in_maps.append({"xT": xT, **packs[s]})
    return in_maps


CONFIG = dict(loop_order="k", do_dedupe=True, do_slim=True,
              x_chunked=True, y_split=False, strassen_p1=True,
              kq_splits=None)

_NC_CACHE = {}


def _get_module():
    key = str(sorted(CONFIG.items()))
    if key not in _NC_CACHE:
        _NC_CACHE[key] = build_module(HIDDEN, INTER // TP, (B * S) // DP,
                                      **CONFIG)
    return _NC_CACHE[key]


def kernel(x, gate_wq, gate_scale, up_wq, up_scale, down_wq, down_scale,
           _return_results=False):
    x = np.asarray(x)
    x_flat = x.reshape(B * S, HIDDEN)
    in_maps = prep_core_inputs(
        x_flat, np.asarray(gate_wq), np.asarray(gate_scale),
        np.asarray(up_wq), np.asarray(up_scale), np.asarray(down_wq),
        HIDDEN, INTER, DP, TP)

    nc = _get_module()
    res = run_bass_kernel_spmd(nc, in_maps, list(range(N_CORES)))

    m = (B * S) // DP
    OH = HIDDEN // P
    y = np.empty((B * S, HIDDEN), np.float32)
    ds = np.asarray(down_scale).astype(np.float32)
    for g in range(DP):
        acc = None
        for s in range(TP):
            part = res.results[g * TP + s]["y"]  # [(KQ*)OH, P, m]
            if part.shape[0] != OH:              # y_split: sum kq partials
                part = part.reshape(-1, OH, P, m).sum(axis=0)
            acc = part if acc is None else acc + part
        # [OH, P, m] -> [hidden, m] -> [m, hidden]
        y[g * m:(g + 1) * m] = acc.reshape(HIDDEN, m).T
    y *= ds[None, :]
    out = y.reshape(B, S, HIDDEN)
    if _return_results:
        return out, res
    return out

